# revision 7
# baseline (speedup 1.0000x reference)
"""Trainium2 Bass kernel for nn_Attention_8924942041930 (sparse_attention).

Reference computation (per batch of N=384 tokens = [t(64) | it(64) | s(256)]):
  qkv = x @ w_qkv
  mix attention: (t+s) queries over all N keys
  inherent attention: it queries over it keys only
  out = concat(t, it, s) @ w_proj + b_proj

Sharding: data-parallel over batch, 8 batches per NeuronCore (64 total / 8).

Per-core dataflow (all matmuls bf16 with f32 PSUM accumulation):
  - host supplies x^T [768, 3072] bf16 so the f-contraction lands on SBUF
    partitions with no on-chip transposes
  - qk^T = w_qkv^T @ x^T per batch -> [hd, token] per head (head pair per
    128-partition chunk: even head rows 0-63, odd head rows 64-127)
  - v = x @ w_v -> [token, hd] per head (PV lhsT needs token on partitions)
  - S^T[k, q] = k^T.T @ q^T per (head, k-chunk); K=64 so head pairs are
    row-packed in the PE array via tile_position rows 0/64 (MMs emitted
    A/B-interleaved so they run concurrently)
  - attn^T = exp(S^T * scale) on ScalarE (logits are tiny: no max-subtract)
  - ts^T = v.T @ attn^T col-packed per head pair (tile_position cols 0/64)
    so the pair lands stacked [128, q] = the proj rhs chunk layout
  - denominators = ones[:,0:64].T @ attn^T as M=64 matmuls per head pair
    (A at cols 0:64, B at 64:128 of the PE array): the PSUM result rows
    0:64 / 64:128 are the A/B denominators replicated 64x, i.e. already in
    the partition-broadcast layout the normalize needs; inherent sums land
    in the same bank at cols N:N+64
  - normalize: fast reciprocal (DVE, PSUM->SBUF) + DVE mult -- no DRAM
    bounce, no broadcast DMAs
  - inherent attention reuses S^T chunk 0 rows/cols 64:128 (it x it block)
  - out^T = w_proj^T @ attnout^T + b_proj (ScalarE Identity+bias), staged
    and stored bf16 -> host converts to f32 and transposes back

Scheduling: the PE instruction queue is strict FIFO, and the attention
matmuls are gated by ScalarE exp throughput, so qk/v chains of batch b+1
and proj chains of batch b-1 are emitted interleaved between the attention
groups of batch b as PE filler work.  Batch 0's attention starts after
just the two chains its first S pair needs (qk0/qk6); the rest of batch
0's chains are that attention's fillers, ordered so every pair's inputs
are emitted before the pair's matmuls (FIFO deadlock otherwise).  All
PSUM tiles are exactly one 2KB bank ([128,512] f32, data in the low
cols): non-bank-aligned tiles straddle banks, which breaks accumulation
-group bank-clear semantics and the simulator's pending-zero model.
Measured pitfalls (ntff traces): every accumulation-group-start LDWEIGHTS
carries the group's semaphore waits (move_matmul_waits_to_ldweights) and
cannot overlap in-flight matmuls, costing ~its own duration (~95-107ns x
~870 groups ~= 40us); packed (tile_position) pairs expose the same cost
on the pair leader only.  Setting InstMatmult.ldweights=False does NOT
suppress the generated LDWEIGHTS.  The chip has a power state that runs
everything at 2.0 instead of 2.4 GHz (check MATMUL avg dur ~320 vs ~384
in the trace before trusting a measurement).
"""

import sys

import numpy as np

if "/opt/trn_rl_repo" not in sys.path:
    sys.path.insert(0, "/opt/trn_rl_repo")

import ml_dtypes

B = 64
N = 384
DIM = 768
H = 12
HD = 64
T_SIZE = 64
S_SIZE = 256
SCALE = HD ** -0.5
NCORES = 8
BPC = B // NCORES  # batches per core
FCH = DIM // 128  # feature chunks of 128
P = 128
NW = N + T_SIZE  # sums/bcast width: mix cols 0:N, inherent cols N:N+64

BF16 = ml_dtypes.bfloat16


def build(n_batches=BPC, sim_safe=False):
    """Build the per-core Bass program (SPMD: same NEFF, per-core data)."""
    import concourse.mybir as mybir
    import concourse.tile as tile
    from concourse import bacc
    from collections import deque
    from contextlib import ExitStack

    bf16 = mybir.dt.bfloat16
    f32 = mybir.dt.float32
    Exp = mybir.ActivationFunctionType.Exp
    Identity = mybir.ActivationFunctionType.Identity
    mult = mybir.AluOpType.mult
    add = mybir.AluOpType.add
    ntok = n_batches * N

    nc = bacc.Bacc("TRN2", target_bir_lowering=False, debug=False,
                   num_devices=NCORES)
    xT = nc.dram_tensor("xT", [DIM, ntok], bf16, kind="ExternalInput")
    wqkv = nc.dram_tensor("wqkv", [DIM, 3 * DIM], bf16, kind="ExternalInput")
    wproj = nc.dram_tensor("wproj", [DIM, DIM], bf16, kind="ExternalInput")
    bproj = nc.dram_tensor("bproj", [DIM], f32, kind="ExternalInput")
    outT = nc.dram_tensor("outT", [DIM, ntok], bf16, kind="ExternalOutput")

    xT_r = xT.rearrange("(o p) t -> p o t", p=P)
    wqkv_r = wqkv.rearrange("(o p) c -> p o c", p=P)
    wproj_r = wproj.rearrange("(o p) c -> p o c", p=P)
    bproj_r = bproj.rearrange("(o p) -> p o", p=P)
    outT_r = outT.rearrange("(o p) t -> p o t", p=P)

    with tile.TileContext(nc) as tc, ExitStack() as ctx:
        const = ctx.enter_context(tc.tile_pool(name="const", bufs=1))
        qk_pool = ctx.enter_context(tc.tile_pool(name="qk", bufs=2))
        v_pool = ctx.enter_context(tc.tile_pool(name="v", bufs=2))
        attn_pool = ctx.enter_context(tc.tile_pool(name="attn", bufs=8))
        pre_pool = ctx.enter_context(tc.tile_pool(name="pre", bufs=4))
        ao_pool = ctx.enter_context(tc.tile_pool(name="ao", bufs=2))
        rec_pool = ctx.enter_context(tc.tile_pool(name="rec", bufs=4))
        out_pool = ctx.enter_context(tc.tile_pool(name="outp", bufs=2))
        # PSUM budget (8 banks): 3 for the dense qk/v/proj chains (freed
        # by DVE/ScalarE copies), 5 for attention S/sums/pv tiles (freed
        # by exp / recip / normalize) -- separating the pools keeps a lag
        # in one consumer class from stalling the other's producers
        chain_ps = ctx.enter_context(tc.tile_pool(name="chainps", bufs=4,
                                                  space="PSUM"))
        work_ps = ctx.enter_context(tc.tile_pool(name="workps", bufs=4,
                                                 space="PSUM"))
        pv_ps = work_ps

        # --- persistent tensors (per-chunk tiles => finer DMA deps) ---
        xT_ch = [const.tile([P, ntok], bf16, tag=f"xT{o}", name=f"xT{o}")
                 for o in range(FCH)]
        wqkv_ch = [const.tile([P, 3 * DIM], bf16, tag=f"wqkv{o}",
                              name=f"wqkv{o}") for o in range(FCH)]
        wproj_ch = [const.tile([P, DIM], bf16, tag=f"wproj{o}",
                               name=f"wproj{o}") for o in range(FCH)]
        # phase 1: the exact slices chain cc=0 needs (xT batch 0 + first
        # wqkv column chunk), spread over all five engine DMA queues so
        # the first matmul can start as early as possible
        qs = [nc.sync, nc.gpsimd, nc.scalar]
        # phase 1: what chains qk0 / qk6 need (xT batch 0 + the two first
        # 128-col weight slices), then qk1/qk7's slices, then w_v, then
        # the rest -- matching the prologue/filler consumption order
        for o in range(FCH):
            qs[o % 3].dma_start(xT_ch[o][:, 0:N], xT_r[:, o, 0:N])
            qs[(o + 1) % 3].dma_start(wqkv_ch[o][:, 0:P], wqkv_r[:, o, 0:P])
        for o in range(FCH):
            qs[o % 3].dma_start(wqkv_ch[o][:, DIM:DIM + P],
                                wqkv_r[:, o, DIM:DIM + P])
            qs[(o + 1) % 3].dma_start(wqkv_ch[o][:, P:N], wqkv_r[:, o, P:N])
        for o in range(FCH):
            qs[o % 3].dma_start(wqkv_ch[o][:, DIM + P:DIM + N],
                                wqkv_r[:, o, DIM + P:DIM + N])
            qs[(o + 1) % 3].dma_start(wqkv_ch[o][:, 2 * DIM:3 * DIM],
                                      wqkv_r[:, o, 2 * DIM:3 * DIM])
        # phase 2: remaining q/k weight columns
        for o in range(FCH):
            qs[o % 3].dma_start(wqkv_ch[o][:, N:DIM], wqkv_r[:, o, N:DIM])
            qs[(o + 1) % 3].dma_start(wqkv_ch[o][:, DIM + N:2 * DIM],
                                      wqkv_r[:, o, DIM + N:2 * DIM])
        # phase 3: remaining activations and proj weights
        q = (ntok - N) // 3 if n_batches > 1 else 0
        for piece in range(3 if q else 0):
            for o in range(FCH):
                eng = nc.sync if o % 2 == 0 else nc.gpsimd
                eng.dma_start(xT_ch[o][:, N + piece * q:N + (piece + 1) * q],
                              xT_r[:, o, N + piece * q:N + (piece + 1) * q])
        for o in range(FCH):
            nc.sync.dma_start(wproj_ch[o][:], wproj_r[:, o, :])
        bproj_sb = const.tile([P, FCH], f32, tag="bproj")
        nc.sync.dma_start(bproj_sb[:], bproj_r[:])
        ones_sb = const.tile([P, 64], bf16, tag="ones")
        nc.gpsimd.memset(ones_sb[:], 1.0)

        qkTs = {}   # b -> qkT tile
        v_sbs = {}  # b -> v tile
        aos = {}    # b -> attnoutT tile

        def qkv_chains(b, order=None):
            """Yield one emitter per accumulation chain, in `order`."""
            t0 = b * N
            qkT = qk_pool.tile([P, 2 * FCH, N], bf16, tag="qkT")
            qkTs[b] = qkT

            def qk_chain(cc):
                def emit():
                    ps = chain_ps.tile([P, 512], mybir.dt.float32,
                                       tag="cw")
                    for fo in range(FCH):
                        nc.tensor.matmul(
                            ps[:, 0:N],
                            wqkv_ch[fo][:, cc * P:(cc + 1) * P],
                            xT_ch[fo][:, t0:t0 + N],
                            start=(fo == 0), stop=(fo == FCH - 1),
                        )
                    nc.vector.tensor_copy(qkT[:, cc, :], ps[:, 0:N])
                return emit

            emitters = {("qk", cc): qk_chain(cc) for cc in range(2 * FCH)}

            v_sb = v_pool.tile([P, 3, H, HD], bf16, tag="v")
            v_sbs[b] = v_sb

            def v_chain(tch, half):
                def emit():
                    ps = chain_ps.tile([P, 512], mybir.dt.float32,
                                       tag="cw")
                    for fo in range(FCH):
                        nc.tensor.matmul(
                            ps[:, 0:N],
                            xT_ch[fo][:, t0 + tch * P:t0 + (tch + 1) * P],
                            wqkv_ch[fo][:,
                                         2 * DIM + half * N:
                                         2 * DIM + (half + 1) * N],
                            start=(fo == 0), stop=(fo == FCH - 1),
                        )
                    nc.vector.tensor_copy(
                        v_sb[:, tch, half * 6:(half + 1) * 6, :],
                        ps[:, 0:N].rearrange("p (h d) -> p h d", d=HD),
                    )
                return emit

            for tch in range(3):
                for half in range(2):
                    emitters[("v", tch, half)] = v_chain(tch, half)
            if order is None:
                order = ([("qk", cc) for cc in range(2 * FCH)]
                         + [("v", t, h) for t in range(3)
                            for h in range(2)])
            for key in order:
                yield emitters[key]

        def proj_chains(b):
            """Yield one emitter per proj output chunk (6) + the store."""
            t0 = b * N
            ao = aos.pop(b)
            outstage = out_pool.tile([P, FCH, N], bf16, tag="outs")

            def proj_chain(cc):
                def emit():
                    ps = chain_ps.tile([P, 512], mybir.dt.float32,
                                       tag="cw")
                    for fo in range(FCH):
                        nc.tensor.matmul(
                            ps[:, 0:N],
                            wproj_ch[fo][:, cc * P:(cc + 1) * P],
                            ao[:, fo, :],
                            start=(fo == 0), stop=(fo == FCH - 1),
                        )
                    nc.vector.tensor_scalar_add(outstage[:, cc, :],
                                                ps[:, 0:N],
                                                bproj_sb[:, cc:cc + 1])
                    eng = nc.sync if cc % 2 == 0 else nc.gpsimd
                    eng.dma_start(outT_r[:, cc, t0:t0 + N],
                                  outstage[:, cc, :])
                return emit

            for cc in range(FCH):
                yield proj_chain(cc)

        def emit_attention(b, fillers):
            """Attention for batch b; pops PE filler chains at stall points."""
            qkT = qkTs.pop(b)
            v_sb = v_sbs[b]

            def fill(k):
                for _ in range(k):
                    if fillers:
                        fillers.popleft()()

            attnoutT = ao_pool.tile([P, FCH, N], bf16, tag="aoT")
            aos[b] = attnoutT
            NP = N - T_SIZE  # 320: packed (t|s) mix-query count

            for g in range(3):  # head groups of 4 (two pairs)
                heads = [4 * g + i for i in range(4)]
                attnTs = {}
                for h in heads:
                    attnTs[h] = attn_pool.tile([P, 3, N], bf16, tag="attnT",
                                               name=f"attnT_{h}")
                # S^T + exp: pairs row-packed; A/B interleaved per kc so the
                # two K=64 matmuls run concurrently in the array.  kc=0
                # computes all N queries (the it x it block feeds the
                # inherent attention); kc=1,2 skip the it queries and pack
                # (t|s) into cols 0:NP -- less PE stream, less exp.
                for pair in range(2):
                    hA, hB = heads[2 * pair], heads[2 * pair + 1]
                    j = hA // 2
                    for kc in range(3):
                        sA = work_ps.tile([P, 512], mybir.dt.float32,
                                          tag="w", name="sA")
                        sB = work_ps.tile([P, 512], mybir.dt.float32,
                                          tag="w", name="sB")
                        kslc = slice(kc * P, (kc + 1) * P)
                        if kc == 0:
                            nc.tensor.matmul(
                                sA[:, 0:N], qkT[0:64, FCH + j, kslc],
                                qkT[0:64, j, :], start=True, stop=True,
                                tile_position=(0, 0))
                            nc.tensor.matmul(
                                sB[:, 0:N], qkT[64:128, FCH + j, kslc],
                                qkT[64:128, j, :], start=True, stop=True,
                                tile_position=(64, 0))
                            nc.scalar.activation(attnTs[hA][:, 0, :],
                                                 sA[:, 0:N], Exp,
                                                 scale=SCALE)
                            nc.scalar.activation(attnTs[hB][:, 0, :],
                                                 sB[:, 0:N], Exp,
                                                 scale=SCALE)
                        else:
                            nc.tensor.matmul(
                                sA[:, 0:T_SIZE], qkT[0:64, FCH + j, kslc],
                                qkT[0:64, j, 0:T_SIZE],
                                start=True, stop=True,
                                tile_position=(0, 0))
                            nc.tensor.matmul(
                                sB[:, 0:T_SIZE], qkT[64:128, FCH + j, kslc],
                                qkT[64:128, j, 0:T_SIZE],
                                start=True, stop=True,
                                tile_position=(64, 0))
                            nc.tensor.matmul(
                                sA[:, T_SIZE:NP], qkT[0:64, FCH + j, kslc],
                                qkT[0:64, j, 2 * T_SIZE:N],
                                start=True, stop=True,
                                tile_position=(0, 0),
                                skip_group_check=True)
                            nc.tensor.matmul(
                                sB[:, T_SIZE:NP], qkT[64:128, FCH + j, kslc],
                                qkT[64:128, j, 2 * T_SIZE:N],
                                start=True, stop=True,
                                tile_position=(64, 0),
                                skip_group_check=True)
                            nc.scalar.activation(attnTs[hA][:, kc, 0:NP],
                                                 sA[:, 0:NP], Exp,
                                                 scale=SCALE)
                            nc.scalar.activation(attnTs[hB][:, kc, 0:NP],
                                                 sB[:, 0:NP], Exp,
                                                 scale=SCALE)
                        # cover the exp-paced s-slot recycle with dense work
                        fill(1)

                for pair in range(2):
                    hA, hB = heads[2 * pair], heads[2 * pair + 1]
                    jc = hA // 2
                    attnA, attnB = attnTs[hA], attnTs[hB]

                    # mix-denominator pre-add over the 3 key chunks on
                    # GpSimd (idle engine): turns the 3-matmul sums chain
                    # into a single N=320 stream on the PE
                    preA = pre_pool.tile([P, NP], bf16, tag="pre",
                                         name="preA")
                    preB = pre_pool.tile([P, NP], bf16, tag="pre",
                                         name="preB")
                    for pre, att in ((preA, attnA), (preB, attnB)):
                        nc.gpsimd.tensor_tensor(
                            pre[:], att[:, 1, 0:NP], att[:, 2, 0:NP], add)
                        nc.gpsimd.tensor_tensor(
                            pre[:, 0:T_SIZE], pre[:, 0:T_SIZE],
                            att[:, 0, 0:T_SIZE], add)
                        nc.gpsimd.tensor_tensor(
                            pre[:, T_SIZE:NP], pre[:, T_SIZE:NP],
                            att[:, 0, 2 * T_SIZE:N], add)

                    # denominators, pair col-packed at M=64: PSUM rows 0:64
                    # get A's sums replicated 64x, rows 64:128 get B's --
                    # i.e. the partition-broadcast layout the TTs need.
                    # mix at cols 0:NP, inherent at cols NP:N.
                    sums = work_ps.tile([P, 512], mybir.dt.float32,
                                        tag="w", name="sums")
                    if sim_safe:
                        nc.vector.memset(sums[:, 0:N], 1.0)
                    nc.tensor.matmul(
                        sums[0:64, 0:NP], ones_sb[:, 0:64], preA[:],
                        start=True, stop=True, tile_position=(0, 0),
                    )
                    nc.tensor.matmul(
                        sums[64:128, 0:NP], ones_sb[:, 0:64], preB[:],
                        start=True, stop=True, tile_position=(0, 64),
                        skip_group_check=True,
                    )
                    nc.tensor.matmul(
                        sums[0:64, NP:N], ones_sb[64:128, 0:64],
                        attnA[64:128, 0, 64:128],
                        start=True, stop=True, tile_position=(64, 0),
                        skip_group_check=True,
                    )
                    nc.tensor.matmul(
                        sums[64:128, NP:N], ones_sb[64:128, 0:64],
                        attnB[64:128, 0, 64:128],
                        start=True, stop=True, tile_position=(64, 64),
                        skip_group_check=True,
                    )
                    recips = rec_pool.tile([P, N], mybir.dt.float32,
                                           tag="rec")
                    nc.vector.reciprocal_approx_fast(recips[:],
                                                     sums[:, 0:N])

                    # PV: inherent (cols NP:N) + mix (cols 0:NP, packed
                    # t|s) in one bank per pair; inherent first since it
                    # only depends on exp(kc0)
                    pv = pv_ps.tile([P, 512], mybir.dt.float32, tag="w",
                                    name="pv")
                    nc.tensor.matmul(
                        pv[0:64, NP:N], v_sb[64:128, 0, hA, :],
                        attnA[64:128, 0, 64:128],
                        start=True, stop=True, tile_position=(64, 0),
                    )
                    nc.tensor.matmul(
                        pv[64:128, NP:N], v_sb[64:128, 0, hB, :],
                        attnB[64:128, 0, 64:128],
                        start=True, stop=True, tile_position=(64, 64),
                        skip_group_check=True,
                    )
                    # PV mix: pair col-packed -> [128, NP] stacked ts^T.
                    # kc1 leads with start=True covering the full 0:NP in
                    # one matmul (the sim's pending-zero model needs each
                    # later matmul's bytes uniformly non-pending); the kc0
                    # t/s slices of the full-N exp tile and kc2 accumulate.
                    nc.tensor.matmul(
                        pv[0:64, 0:NP], v_sb[:, 1, hA, :],
                        attnA[:, 1, 0:NP],
                        start=True, stop=False, tile_position=(0, 0),
                    )
                    nc.tensor.matmul(
                        pv[64:128, 0:NP], v_sb[:, 1, hB, :],
                        attnB[:, 1, 0:NP],
                        start=True, stop=False, tile_position=(0, 64),
                        skip_group_check=True,
                    )
                    nc.tensor.matmul(
                        pv[0:64, 0:T_SIZE], v_sb[:, 0, hA, :],
                        attnA[:, 0, 0:T_SIZE],
                        start=False, stop=False, tile_position=(0, 0),
                    )
                    nc.tensor.matmul(
                        pv[64:128, 0:T_SIZE], v_sb[:, 0, hB, :],
                        attnB[:, 0, 0:T_SIZE],
                        start=False, stop=False, tile_position=(0, 64),
                        skip_group_check=True,
                    )
                    nc.tensor.matmul(
                        pv[0:64, T_SIZE:NP], v_sb[:, 0, hA, :],
                        attnA[:, 0, 2 * T_SIZE:N],
                        start=False, stop=False, tile_position=(0, 0),
                    )
                    nc.tensor.matmul(
                        pv[64:128, T_SIZE:NP], v_sb[:, 0, hB, :],
                        attnB[:, 0, 2 * T_SIZE:N],
                        start=False, stop=False, tile_position=(0, 64),
                        skip_group_check=True,
                    )
                    nc.tensor.matmul(
                        pv[0:64, 0:NP], v_sb[:, 2, hA, :],
                        attnA[:, 2, 0:NP],
                        start=False, stop=True, tile_position=(0, 0),
                    )
                    nc.tensor.matmul(
                        pv[64:128, 0:NP], v_sb[:, 2, hB, :],
                        attnB[:, 2, 0:NP],
                        start=False, stop=True, tile_position=(0, 64),
                        skip_group_check=True,
                    )
                    nc.vector.tensor_tensor(
                        attnoutT[:, jc, 0:T_SIZE], pv[:, 0:T_SIZE],
                        recips[:, 0:T_SIZE], mult)
                    nc.vector.tensor_tensor(
                        attnoutT[:, jc, 2 * T_SIZE:N], pv[:, T_SIZE:NP],
                        recips[:, T_SIZE:NP], mult)
                    nc.vector.tensor_tensor(
                        attnoutT[:, jc, T_SIZE:2 * T_SIZE], pv[:, NP:N],
                        recips[:, NP:N], mult)
                    fill(1)

        # prologue: only the chains attention(0)'s first S pair needs
        # (q chunk 0, k chunk 6, v for its PV); the rest of batch 0's
        # chains are emitted as that attention's fillers, ordered so each
        # pair's inputs are emitted before the pair's matmuls (the PE
        # queue is strict FIFO -- emitting a consumer before its producer
        # would deadlock)
        order0 = ([("qk", 0), ("qk", 6)],
                  [("qk", 1), ("qk", 7),
                   ("v", 0, 0), ("v", 1, 0), ("v", 2, 0),
                   ("qk", 2), ("qk", 8), ("v", 0, 1),
                   ("qk", 3), ("qk", 9), ("v", 1, 1), ("v", 2, 1),
                   ("qk", 4), ("qk", 10), ("qk", 5), ("qk", 11)])
        chain0 = list(qkv_chains(0, order=order0[0] + order0[1]))
        for emit in chain0[:len(order0[0])]:
            emit()
        for b in range(n_batches):
            fillers = deque()
            if b == 0:
                fillers.extend(chain0[len(order0[0]):])
            if b + 1 < n_batches:
                fillers.extend(qkv_chains(b + 1))
            if b >= 1:
                fillers.extend(proj_chains(b - 1))
            emit_attention(b, fillers)
            while fillers:
                fillers.popleft()()
        for emit in proj_chains(n_batches - 1):
            emit()

    nc.compile()
    return nc


_CACHED_NC = None


def _get_nc():
    global _CACHED_NC
    if _CACHED_NC is None:
        _CACHED_NC = build(BPC)
    return _CACHED_NC


def kernel(x, w_qkv, w_proj, b_proj):
    from concourse.bass_utils import run_bass_kernel_spmd

    nc = _get_nc()

    wqkv_bf = np.ascontiguousarray(w_qkv.astype(BF16))
    wproj_bf = np.ascontiguousarray(w_proj.astype(BF16))
    bproj_f = np.ascontiguousarray(b_proj.astype(np.float32))

    in_maps = []
    for c in range(NCORES):
        xc = x[c * BPC:(c + 1) * BPC].reshape(BPC * N, DIM)
        xT = np.ascontiguousarray(xc.T.astype(BF16))
        in_maps.append({
            "xT": xT,
            "wqkv": wqkv_bf,
            "wproj": wproj_bf,
            "bproj": bproj_f,
        })

    res = run_bass_kernel_spmd(nc, in_maps, core_ids=list(range(NCORES)))
    outs = [
        np.ascontiguousarray(
            res.results[c]["outT"].astype(np.float32).T,
        ).reshape(BPC, N, DIM)
        for c in range(NCORES)
    ]
    return np.concatenate(outs, axis=0)


if __name__ == "__main__":
    rng = np.random.default_rng(0)
    x = rng.standard_normal((B, N, DIM), dtype=np.float32)
    w_qkv = (rng.standard_normal((DIM, 3 * DIM), dtype=np.float32) * 0.02)
    w_proj = (rng.standard_normal((DIM, DIM), dtype=np.float32) * 0.02)
    b_proj = np.zeros((DIM,), dtype=np.float32)
    out = kernel(x, w_qkv, w_proj, b_proj)
    print("out", out.shape, out.dtype, float(np.abs(out).max()))



# revision 15
# speedup vs baseline: 1.0730x; 1.0730x over previous
"""Trainium2 Bass kernel for nn_Attention_8924942041930 (sparse_attention).

Reference computation (per batch of N=384 tokens = [t(64) | it(64) | s(256)]):
  qkv = x @ w_qkv
  mix attention: (t+s) queries over all N keys
  inherent attention: it queries over it keys only
  out = concat(t, it, s) @ w_proj + b_proj

Sharding: data-parallel over batch, 8 batches per NeuronCore (64 total / 8).

Per-core dataflow (all matmuls bf16 with f32 PSUM accumulation):
  - host supplies x^T [768, 3072] bf16 so the f-contraction lands on SBUF
    partitions with no on-chip transposes
  - qk^T = w_qkv^T @ x^T per batch -> [hd, token] per head (head pair per
    128-partition chunk: even head rows 0-63, odd head rows 64-127)
  - v = x @ w_v -> [token, hd] per head (PV lhsT needs token on partitions)
  - S^T[k, q] = k^T.T @ q^T per (head, k-chunk); K=64 so head pairs are
    row-packed in the PE array via tile_position rows 0/64 (MMs emitted
    A/B-interleaved so they run concurrently)
  - attn^T = exp(S^T * scale) on ScalarE (logits are tiny: no max-subtract)
  - ts^T = v.T @ attn^T col-packed per head pair (tile_position cols 0/64)
    so the pair lands stacked [128, q] = the proj rhs chunk layout
  - denominators = ones[:,0:64].T @ attn^T as M=64 matmuls per head pair
    (A at cols 0:64, B at 64:128 of the PE array): the PSUM result rows
    0:64 / 64:128 are the A/B denominators replicated 64x, i.e. already in
    the partition-broadcast layout the normalize needs; inherent sums land
    in the same bank at cols N:N+64
  - normalize: fast reciprocal (DVE, PSUM->SBUF) + DVE mult -- no DRAM
    bounce, no broadcast DMAs
  - inherent attention reuses S^T chunk 0 rows/cols 64:128 (it x it block)
  - out^T = w_proj^T @ attnout^T + b_proj (ScalarE Identity+bias), staged
    and stored bf16 -> host converts to f32 and transposes back

Scheduling: the PE instruction queue is strict FIFO, and the attention
matmuls are gated by ScalarE exp throughput, so qk/v chains of batch b+1
and proj chains of batch b-1 are emitted interleaved between the attention
groups of batch b as PE filler work.  Batch 0's attention starts after
just the two chains its first S pair needs (qk0/qk6); the rest of batch
0's chains are that attention's fillers, ordered so every pair's inputs
are emitted before the pair's matmuls (FIFO deadlock otherwise).  All
PSUM tiles are exactly one 2KB bank ([128,512] f32, data in the low
cols): non-bank-aligned tiles straddle banks, which breaks accumulation
-group bank-clear semantics and the simulator's pending-zero model.
Measured pitfalls (ntff traces): every accumulation-group-start LDWEIGHTS
carries the group's semaphore waits (move_matmul_waits_to_ldweights) and
cannot overlap in-flight matmuls, costing ~its own duration (~95-107ns x
~870 groups ~= 40us); packed (tile_position) pairs expose the same cost
on the pair leader only.  Setting InstMatmult.ldweights=False does NOT
suppress the generated LDWEIGHTS.  The chip has a power state that runs
everything at 2.0 instead of 2.4 GHz (check MATMUL avg dur ~320 vs ~384
in the trace before trusting a measurement).
"""

import sys

import numpy as np

if "/opt/trn_rl_repo" not in sys.path:
    sys.path.insert(0, "/opt/trn_rl_repo")

import ml_dtypes

B = 64
N = 384
DIM = 768
H = 12
HD = 64
T_SIZE = 64
S_SIZE = 256
SCALE = HD ** -0.5
NCORES = 8
BPC = B // NCORES  # batches per core
FCH = DIM // 128  # feature chunks of 128
P = 128
NW = N + T_SIZE  # sums/bcast width: mix cols 0:N, inherent cols N:N+64

BF16 = ml_dtypes.bfloat16


def build(n_batches=BPC, sim_safe=False):
    """Build the per-core Bass program (SPMD: same NEFF, per-core data)."""
    import concourse.mybir as mybir
    import concourse.tile as tile
    from concourse import bacc
    from collections import deque
    from contextlib import ExitStack

    bf16 = mybir.dt.bfloat16
    f32 = mybir.dt.float32
    Exp = mybir.ActivationFunctionType.Exp
    Identity = mybir.ActivationFunctionType.Identity
    mult = mybir.AluOpType.mult
    add = mybir.AluOpType.add
    ntok = n_batches * N

    nc = bacc.Bacc("TRN2", target_bir_lowering=False, debug=False,
                   num_devices=NCORES)
    xT = nc.dram_tensor("xT", [DIM, ntok], bf16, kind="ExternalInput")
    wqkv = nc.dram_tensor("wqkv", [DIM, 3 * DIM], bf16, kind="ExternalInput")
    wproj = nc.dram_tensor("wproj", [DIM, DIM], bf16, kind="ExternalInput")
    bproj = nc.dram_tensor("bproj", [DIM], f32, kind="ExternalInput")
    outT = nc.dram_tensor("outT", [DIM, ntok], bf16, kind="ExternalOutput")

    xT_r = xT.rearrange("(o p) t -> p o t", p=P)
    wqkv_r = wqkv.rearrange("(o p) c -> p o c", p=P)
    wproj_r = wproj.rearrange("(o p) c -> p o c", p=P)
    bproj_r = bproj.rearrange("(o p) -> p o", p=P)
    outT_r = outT.rearrange("(o p) t -> p o t", p=P)

    with tile.TileContext(nc) as tc, ExitStack() as ctx:
        const = ctx.enter_context(tc.tile_pool(name="const", bufs=1))
        qk_pool = ctx.enter_context(tc.tile_pool(name="qk", bufs=2))
        v_pool = ctx.enter_context(tc.tile_pool(name="v", bufs=2))
        attn_pool = ctx.enter_context(tc.tile_pool(name="attn", bufs=8))
        ao_pool = ctx.enter_context(tc.tile_pool(name="ao", bufs=2))
        rec_pool = ctx.enter_context(tc.tile_pool(name="rec", bufs=4))
        out_pool = ctx.enter_context(tc.tile_pool(name="outp", bufs=2))
        # PSUM budget (8 banks): 3 for the dense qk/v/proj chains (freed
        # by DVE/ScalarE copies), 5 for attention S/sums/pv tiles (freed
        # by exp / recip / normalize) -- separating the pools keeps a lag
        # in one consumer class from stalling the other's producers
        chain_ps = ctx.enter_context(tc.tile_pool(name="chainps", bufs=4,
                                                  space="PSUM"))
        work_ps = ctx.enter_context(tc.tile_pool(name="workps", bufs=4,
                                                 space="PSUM"))
        pv_ps = work_ps

        # --- persistent tensors (per-chunk tiles => finer DMA deps) ---
        xT_ch = [const.tile([P, ntok], bf16, tag=f"xT{o}", name=f"xT{o}")
                 for o in range(FCH)]
        wqkv_ch = [const.tile([P, 3 * DIM], bf16, tag=f"wqkv{o}",
                              name=f"wqkv{o}") for o in range(FCH)]
        wproj_ch = [const.tile([P, DIM], bf16, tag=f"wproj{o}",
                               name=f"wproj{o}") for o in range(FCH)]
        # phase 1: exactly what the prologue pair (qk 0 / qk 6) streams,
        # in matmul order, round-robin over four engine DMA queues so the
        # first chain's inputs land as early and evenly as possible
        qs3 = [nc.sync, nc.gpsimd, nc.scalar]
        qi = [0]

        def dma_rr(dst, src):
            qs3[qi[0] % 3].dma_start(dst, src)
            qi[0] += 1

        for o in range(FCH):
            dma_rr(xT_ch[o][:, 0:N], xT_r[:, o, 0:N])
            dma_rr(wqkv_ch[o][:, 0:P], wqkv_r[:, o, 0:P])
            dma_rr(wqkv_ch[o][:, DIM:DIM + P], wqkv_r[:, o, DIM:DIM + P])
        # phase 1b: the qk(1,7) pair, then the v weights the early v
        # fillers need (both halves)
        for o in range(FCH):
            dma_rr(wqkv_ch[o][:, P:2 * P], wqkv_r[:, o, P:2 * P])
            dma_rr(wqkv_ch[o][:, DIM + P:DIM + 2 * P],
                   wqkv_r[:, o, DIM + P:DIM + 2 * P])
        for o in range(FCH):
            dma_rr(wqkv_ch[o][:, 2 * DIM:3 * DIM],
                   wqkv_r[:, o, 2 * DIM:3 * DIM])
        # phase 2: remaining q/k weight columns (3 queues: keep the DVE
        # queue free for the first chain casts)
        qs = [nc.sync, nc.gpsimd, nc.scalar]
        for o in range(FCH):
            qs[o % 3].dma_start(wqkv_ch[o][:, 2 * P:N],
                                wqkv_r[:, o, 2 * P:N])
            qs[(o + 1) % 3].dma_start(wqkv_ch[o][:, N:DIM],
                                      wqkv_r[:, o, N:DIM])
            qs[(o + 2) % 3].dma_start(wqkv_ch[o][:, DIM + 2 * P:DIM + N],
                                      wqkv_r[:, o, DIM + 2 * P:DIM + N])
            qs[o % 3].dma_start(wqkv_ch[o][:, DIM + N:2 * DIM],
                                wqkv_r[:, o, DIM + N:2 * DIM])
        # phase 3: remaining activations and proj weights
        q = (ntok - N) // 3 if n_batches > 1 else 0
        for piece in range(3 if q else 0):
            for o in range(FCH):
                eng = nc.sync if o % 2 == 0 else nc.gpsimd
                eng.dma_start(xT_ch[o][:, N + piece * q:N + (piece + 1) * q],
                              xT_r[:, o, N + piece * q:N + (piece + 1) * q])
        for o in range(FCH):
            nc.sync.dma_start(wproj_ch[o][:], wproj_r[:, o, :])
        bproj_sb = const.tile([P, FCH], f32, tag="bproj")
        nc.sync.dma_start(bproj_sb[:], bproj_r[:])
        ones_sb = const.tile([P, 64], bf16, tag="ones")
        nc.gpsimd.memset(ones_sb[:], 1.0)

        qkTs = {}   # b -> qkT tile
        v_sbs = {}  # b -> v tile
        aos = {}    # b -> attnoutT tile

        def qkv_chains(b):
            """Yield half-pair emitters: two PE-interleaved chains, each
            split in two 3-matmul parts.  Interleaving keeps every
            ldweights (including the chain-boundary one) loading while the
            partner chain's matmul streams, so no boundary exposure; the
            half-pair split keeps filler granularity at ~6 matmuls."""
            t0 = b * N
            qkT = qk_pool.tile([P, 2 * FCH, N], bf16, tag="qkT")
            qkTs[b] = qkT
            v_sb = v_pool.tile([P, 3, H, HD], bf16, tag="v")
            v_sbs[b] = v_sb

            def qk_pair(ccA, ccB):
                st = {}

                def mm(ps, cc, fo, start, stop, skip):
                    nc.tensor.matmul(
                        ps[:, 0:N], wqkv_ch[fo][:, cc * P:(cc + 1) * P],
                        xT_ch[fo][:, t0:t0 + N],
                        start=start, stop=stop, skip_group_check=skip)

                def p1():
                    psA = chain_ps.tile([P, 512], mybir.dt.float32,
                                        tag="cw")
                    psB = chain_ps.tile([P, 512], mybir.dt.float32,
                                        tag="cw")
                    st["ps"] = (psA, psB)
                    for fo in range(3):
                        mm(psA, ccA, fo, fo == 0, False, False)
                        mm(psB, ccB, fo, fo == 0, False, False)

                def p2():
                    psA, psB = st["ps"]
                    for fo in range(3, FCH):
                        mm(psA, ccA, fo, False, fo == FCH - 1, False)
                        mm(psB, ccB, fo, False, fo == FCH - 1, False)
                    nc.vector.tensor_copy(qkT[:, ccA, :], psA[:, 0:N])
                    nc.vector.tensor_copy(qkT[:, ccB, :], psB[:, 0:N])

                return [p1, p2]

            def v_pair(chA, chB):
                st = {}

                def mm(ps, ch, fo, start, stop, skip):
                    tch, half = ch
                    nc.tensor.matmul(
                        ps[:, 0:N],
                        xT_ch[fo][:, t0 + tch * P:t0 + (tch + 1) * P],
                        wqkv_ch[fo][:, 2 * DIM + half * N:
                                     2 * DIM + (half + 1) * N],
                        start=start, stop=stop, skip_group_check=skip)

                def p1():
                    psA = chain_ps.tile([P, 512], mybir.dt.float32,
                                        tag="cw")
                    psB = chain_ps.tile([P, 512], mybir.dt.float32,
                                        tag="cw")
                    st["ps"] = (psA, psB)
                    for fo in range(3):
                        mm(psA, chA, fo, fo == 0, False, False)
                        mm(psB, chB, fo, fo == 0, False, False)

                def p2():
                    psA, psB = st["ps"]
                    for fo in range(3, FCH):
                        mm(psA, chA, fo, False, fo == FCH - 1, False)
                        mm(psB, chB, fo, False, fo == FCH - 1, False)
                    for ps, (tch, half) in ((psA, chA), (psB, chB)):
                        nc.vector.tensor_copy(
                            v_sb[:, tch, half * 6:(half + 1) * 6, :],
                            ps[:, 0:N].rearrange("p (h d) -> p h d",
                                                 d=HD))

                return [p1, p2]

            # order matters for batch 0 (emission-order FIFO safety vs
            # attention(0)'s fillers); for later batches everything is
            # emitted during the previous attention anyway
            parts = (qk_pair(0, 6) + qk_pair(1, 7)
                     + v_pair((0, 0), (1, 0)) + v_pair((2, 0), (0, 1))
                     + qk_pair(2, 8) + qk_pair(3, 9)
                     + v_pair((1, 1), (2, 1))
                     + qk_pair(4, 10) + qk_pair(5, 11))
            yield from parts

        def proj_chains(b):
            """Yield half-pair emitters for the 6 proj chunks (3 pairs)."""
            t0 = b * N
            ao = aos.pop(b)
            outstage = out_pool.tile([P, FCH, N], bf16, tag="outs")

            def pj_pair(ccA, ccB):
                st = {}

                def mm(ps, cc, fo, start, stop, skip):
                    nc.tensor.matmul(
                        ps[:, 0:N], wproj_ch[fo][:, cc * P:(cc + 1) * P],
                        ao[:, fo, :], start=start, stop=stop,
                        skip_group_check=skip)

                def p1():
                    psA = chain_ps.tile([P, 512], mybir.dt.float32,
                                        tag="cw")
                    psB = chain_ps.tile([P, 512], mybir.dt.float32,
                                        tag="cw")
                    st["ps"] = (psA, psB)
                    for fo in range(3):
                        mm(psA, ccA, fo, fo == 0, False, False)
                        mm(psB, ccB, fo, fo == 0, False, False)

                def p2():
                    psA, psB = st["ps"]
                    for fo in range(3, FCH):
                        mm(psA, ccA, fo, False, fo == FCH - 1, False)
                        mm(psB, ccB, fo, False, fo == FCH - 1, False)
                    for ps, cc in ((psA, ccA), (psB, ccB)):
                        nc.vector.tensor_scalar_add(outstage[:, cc, :],
                                                    ps[:, 0:N],
                                                    bproj_sb[:, cc:cc + 1])
                        eng = nc.sync if cc % 2 == 0 else nc.gpsimd
                        eng.dma_start(outT_r[:, cc, t0:t0 + N],
                                      outstage[:, cc, :])

                return [p1, p2]

            for cc in range(0, FCH, 2):
                yield from pj_pair(cc, cc + 1)

        def emit_attention(b, fillers):
            """Attention for batch b; pops PE filler chains at stall points."""
            qkT = qkTs.pop(b)
            v_sb = v_sbs[b]

            def fill(k):
                for _ in range(k):
                    if fillers:
                        fillers.popleft()()

            attnoutT = ao_pool.tile([P, FCH, N], bf16, tag="aoT")
            aos[b] = attnoutT
            NP = N - T_SIZE  # 320: packed (t|s) mix-query count

            for g in range(3):  # head groups of 4 (two pairs)
                heads = [4 * g + i for i in range(4)]
                attnTs = {}
                for h in heads:
                    attnTs[h] = attn_pool.tile([P, 3, N], bf16, tag="attnT",
                                               name=f"attnT_{h}")
                # S^T + exp: pairs row-packed; A/B interleaved per kc so the
                # two K=64 matmuls run concurrently in the array.  kc=0
                # computes all N queries (the it x it block feeds the
                # inherent attention); kc=1,2 skip the it queries and pack
                # (t|s) into cols 0:NP -- less PE stream, less exp.
                for pair in range(2):
                    hA, hB = heads[2 * pair], heads[2 * pair + 1]
                    j = hA // 2
                    for kc in range(3):
                        sA = work_ps.tile([P, 512], mybir.dt.float32,
                                          tag="w", name="sA")
                        sB = work_ps.tile([P, 512], mybir.dt.float32,
                                          tag="w", name="sB")
                        kslc = slice(kc * P, (kc + 1) * P)
                        if kc == 0:
                            nc.tensor.matmul(
                                sA[:, 0:N], qkT[0:64, FCH + j, kslc],
                                qkT[0:64, j, :], start=True, stop=True,
                                tile_position=(0, 0))
                            nc.tensor.matmul(
                                sB[:, 0:N], qkT[64:128, FCH + j, kslc],
                                qkT[64:128, j, :], start=True, stop=True,
                                tile_position=(64, 0))
                            nc.scalar.activation(attnTs[hA][:, 0, :],
                                                 sA[:, 0:N], Exp,
                                                 scale=SCALE)
                            nc.scalar.activation(attnTs[hB][:, 0, :],
                                                 sB[:, 0:N], Exp,
                                                 scale=SCALE)
                        else:
                            nc.tensor.matmul(
                                sA[:, 0:T_SIZE], qkT[0:64, FCH + j, kslc],
                                qkT[0:64, j, 0:T_SIZE],
                                start=True, stop=True,
                                tile_position=(0, 0))
                            nc.tensor.matmul(
                                sB[:, 0:T_SIZE], qkT[64:128, FCH + j, kslc],
                                qkT[64:128, j, 0:T_SIZE],
                                start=True, stop=True,
                                tile_position=(64, 0))
                            nc.tensor.matmul(
                                sA[:, T_SIZE:NP], qkT[0:64, FCH + j, kslc],
                                qkT[0:64, j, 2 * T_SIZE:N],
                                start=True, stop=True,
                                tile_position=(0, 0),
                                skip_group_check=True)
                            nc.tensor.matmul(
                                sB[:, T_SIZE:NP], qkT[64:128, FCH + j, kslc],
                                qkT[64:128, j, 2 * T_SIZE:N],
                                start=True, stop=True,
                                tile_position=(64, 0),
                                skip_group_check=True)
                            nc.scalar.activation(attnTs[hA][:, kc, 0:NP],
                                                 sA[:, 0:NP], Exp,
                                                 scale=SCALE)
                            nc.scalar.activation(attnTs[hB][:, kc, 0:NP],
                                                 sB[:, 0:NP], Exp,
                                                 scale=SCALE)
                        # cover the exp-paced s-slot recycle with dense work
                        fill(1)

                for pair in range(2):
                    hA, hB = heads[2 * pair], heads[2 * pair + 1]
                    jc = hA // 2
                    attnA, attnB = attnTs[hA], attnTs[hB]

                    # denominators, pair col-packed at M=64: PSUM rows 0:64
                    # get A's sums replicated 64x, rows 64:128 get B's --
                    # i.e. the partition-broadcast layout the TTs need.
                    # mix at cols 0:NP (kc1 leads full-width for the sim's
                    # pending-zero model; kc0 t/s slices and kc2
                    # accumulate), inherent at cols NP:N.
                    sums = work_ps.tile([P, 512], mybir.dt.float32,
                                        tag="w", name="sums")
                    if sim_safe:
                        nc.vector.memset(sums[:, 0:N], 1.0)
                    for rows, att, tp, skip in (
                            (slice(0, 64), attnA, (0, 0), False),
                            (slice(64, 128), attnB, (0, 64), True)):
                        nc.tensor.matmul(
                            sums[rows, 0:NP], ones_sb[:, 0:64],
                            att[:, 1, 0:NP],
                            start=True, stop=False, tile_position=tp,
                            skip_group_check=skip,
                        )
                    for rows, att, tp in ((slice(0, 64), attnA, (0, 0)),
                                          (slice(64, 128), attnB, (0, 64))):
                        nc.tensor.matmul(
                            sums[rows, 0:T_SIZE], ones_sb[:, 0:64],
                            att[:, 0, 0:T_SIZE],
                            start=False, stop=False, tile_position=tp,
                            skip_group_check=True,
                        )
                        nc.tensor.matmul(
                            sums[rows, T_SIZE:NP], ones_sb[:, 0:64],
                            att[:, 0, 2 * T_SIZE:N],
                            start=False, stop=False, tile_position=tp,
                            skip_group_check=True,
                        )
                    for rows, att, tp, skip in (
                            (slice(0, 64), attnA, (0, 0), False),
                            (slice(64, 128), attnB, (0, 64), True)):
                        nc.tensor.matmul(
                            sums[rows, 0:NP], ones_sb[:, 0:64],
                            att[:, 2, 0:NP],
                            start=False, stop=True, tile_position=tp,
                            skip_group_check=skip,
                        )
                    nc.tensor.matmul(
                        sums[0:64, NP:N], ones_sb[64:128, 0:64],
                        attnA[64:128, 0, 64:128],
                        start=True, stop=True, tile_position=(64, 0),
                        skip_group_check=True,
                    )
                    nc.tensor.matmul(
                        sums[64:128, NP:N], ones_sb[64:128, 0:64],
                        attnB[64:128, 0, 64:128],
                        start=True, stop=True, tile_position=(64, 64),
                        skip_group_check=True,
                    )
                    recips = rec_pool.tile([P, N], mybir.dt.float32,
                                           tag="rec")
                    nc.vector.reciprocal_approx_fast(recips[:],
                                                     sums[:, 0:N])

                    # PV: inherent (cols NP:N) + mix (cols 0:NP, packed
                    # t|s) in one bank per pair; inherent first since it
                    # only depends on exp(kc0)
                    pv = pv_ps.tile([P, 512], mybir.dt.float32, tag="w",
                                    name="pv")
                    nc.tensor.matmul(
                        pv[0:64, NP:N], v_sb[64:128, 0, hA, :],
                        attnA[64:128, 0, 64:128],
                        start=True, stop=True, tile_position=(64, 0),
                    )
                    nc.tensor.matmul(
                        pv[64:128, NP:N], v_sb[64:128, 0, hB, :],
                        attnB[64:128, 0, 64:128],
                        start=True, stop=True, tile_position=(64, 64),
                        skip_group_check=True,
                    )
                    # PV mix: pair col-packed -> [128, NP] stacked ts^T.
                    # kc1 leads with start=True covering the full 0:NP in
                    # one matmul (the sim's pending-zero model needs each
                    # later matmul's bytes uniformly non-pending); the kc0
                    # t/s slices of the full-N exp tile and kc2 accumulate.
                    nc.tensor.matmul(
                        pv[0:64, 0:NP], v_sb[:, 1, hA, :],
                        attnA[:, 1, 0:NP],
                        start=True, stop=False, tile_position=(0, 0),
                    )
                    nc.tensor.matmul(
                        pv[64:128, 0:NP], v_sb[:, 1, hB, :],
                        attnB[:, 1, 0:NP],
                        start=True, stop=False, tile_position=(0, 64),
                        skip_group_check=True,
                    )
                    nc.tensor.matmul(
                        pv[0:64, 0:T_SIZE], v_sb[:, 0, hA, :],
                        attnA[:, 0, 0:T_SIZE],
                        start=False, stop=False, tile_position=(0, 0),
                    )
                    nc.tensor.matmul(
                        pv[64:128, 0:T_SIZE], v_sb[:, 0, hB, :],
                        attnB[:, 0, 0:T_SIZE],
                        start=False, stop=False, tile_position=(0, 64),
                        skip_group_check=True,
                    )
                    nc.tensor.matmul(
                        pv[0:64, T_SIZE:NP], v_sb[:, 0, hA, :],
                        attnA[:, 0, 2 * T_SIZE:N],
                        start=False, stop=False, tile_position=(0, 0),
                    )
                    nc.tensor.matmul(
                        pv[64:128, T_SIZE:NP], v_sb[:, 0, hB, :],
                        attnB[:, 0, 2 * T_SIZE:N],
                        start=False, stop=False, tile_position=(0, 64),
                        skip_group_check=True,
                    )
                    nc.tensor.matmul(
                        pv[0:64, 0:NP], v_sb[:, 2, hA, :],
                        attnA[:, 2, 0:NP],
                        start=False, stop=True, tile_position=(0, 0),
                    )
                    nc.tensor.matmul(
                        pv[64:128, 0:NP], v_sb[:, 2, hB, :],
                        attnB[:, 2, 0:NP],
                        start=False, stop=True, tile_position=(0, 64),
                        skip_group_check=True,
                    )
                    nc.vector.tensor_tensor(
                        attnoutT[:, jc, 0:T_SIZE], pv[:, 0:T_SIZE],
                        recips[:, 0:T_SIZE], mult)
                    nc.vector.tensor_tensor(
                        attnoutT[:, jc, 2 * T_SIZE:N], pv[:, T_SIZE:NP],
                        recips[:, T_SIZE:NP], mult)
                    nc.vector.tensor_tensor(
                        attnoutT[:, jc, T_SIZE:2 * T_SIZE], pv[:, NP:N],
                        recips[:, NP:N], mult)
                    fill(1)

        # prologue: only the chain pair attention(0)'s first S pair needs
        # (q chunk 0 / k chunk 6); the rest of batch 0's chains are that
        # attention's fillers, ordered so each attention group's inputs
        # are emitted before the group's matmuls (the PE queue is strict
        # FIFO -- emitting a consumer before its producer would deadlock)
        chain0 = list(qkv_chains(0))
        for emit in chain0[:2]:
            emit()
        for b in range(n_batches):
            fillers = deque()
            if b == 0:
                fillers.extend(chain0[2:])
            if b + 1 < n_batches:
                fillers.extend(qkv_chains(b + 1))
            if b >= 1:
                fillers.extend(proj_chains(b - 1))
            emit_attention(b, fillers)
            while fillers:
                fillers.popleft()()
        for emit in proj_chains(n_batches - 1):
            emit()

    nc.compile()
    return nc


_CACHED_NC = None


def _get_nc():
    global _CACHED_NC
    if _CACHED_NC is None:
        _CACHED_NC = build(BPC)
    return _CACHED_NC


def kernel(x, w_qkv, w_proj, b_proj):
    from concourse.bass_utils import run_bass_kernel_spmd

    nc = _get_nc()

    wqkv_bf = np.ascontiguousarray(w_qkv.astype(BF16))
    wproj_bf = np.ascontiguousarray(w_proj.astype(BF16))
    bproj_f = np.ascontiguousarray(b_proj.astype(np.float32))

    in_maps = []
    for c in range(NCORES):
        xc = x[c * BPC:(c + 1) * BPC].reshape(BPC * N, DIM)
        xT = np.ascontiguousarray(xc.T.astype(BF16))
        in_maps.append({
            "xT": xT,
            "wqkv": wqkv_bf,
            "wproj": wproj_bf,
            "bproj": bproj_f,
        })

    res = run_bass_kernel_spmd(nc, in_maps, core_ids=list(range(NCORES)))
    outs = [
        np.ascontiguousarray(
            res.results[c]["outT"].astype(np.float32).T,
        ).reshape(BPC, N, DIM)
        for c in range(NCORES)
    ]
    return np.concatenate(outs, axis=0)


if __name__ == "__main__":
    rng = np.random.default_rng(0)
    x = rng.standard_normal((B, N, DIM), dtype=np.float32)
    w_qkv = (rng.standard_normal((DIM, 3 * DIM), dtype=np.float32) * 0.02)
    w_proj = (rng.standard_normal((DIM, DIM), dtype=np.float32) * 0.02)
    b_proj = np.zeros((DIM,), dtype=np.float32)
    out = kernel(x, w_qkv, w_proj, b_proj)
    print("out", out.shape, out.dtype, float(np.abs(out).max()))

